# revision 1
# baseline (speedup 1.0000x reference)
"""Trainium2 Bass kernel for the Dedicom decoder problem.

Math: with U = z * d (row-wise scale by the selected local_diag row),
    score_b = ((z[e0]*d) @ W) * d . z[e1] = U[e0] @ W @ U[e1]^T
so all-pairs scores S = (U @ W) @ U^T  ([N_DRUGS, N_DRUGS]) contain every
edge score.  We shard S by e0-block across the 8 cores: core c computes
S rows [512c, 512c+512) (~2.1 GF in bf16), streams them to DRAM, then a
256B-granular dma_gather pulls each edge's 128-wide candidate block and a
host-built one-hot mask + segmented reduce extracts the scalar, followed
by an on-chip sigmoid.  Edges are bucketed to cores by e0>>9 on the host;
results are scattered back to their original positions on the host.
"""

import numpy as np
import ml_dtypes

BF = ml_dtypes.bfloat16

N_DRUGS = 4096
D = 512
N_CORES = 8
BLK = N_DRUGS // N_CORES  # 512 rows of S per core
KC = D // 128             # 4 contraction chunks
MT = BLK // 128           # 4 row tiles of the core's S block
NCH = N_DRUGS // 512      # 8 column chunks of S
TPB = N_DRUGS // 128      # 32 tokens (128-wide blocks) per S row

_cache = {}


def _build(cap, dep_mode="helper", inplace=True, tail=True, gather_mode="real",
           ms_load=True):
    """Build + compile the SPMD program for a per-core edge capacity `cap`."""
    import concourse.bass as bass  # noqa: F401
    import concourse.bacc as bacc
    import concourse.mybir as mybir
    import concourse.tile as tile
    from concourse.tile import add_dep_helper

    f32 = mybir.dt.float32
    bf16 = mybir.dt.bfloat16
    i16 = mybir.dt.int16
    nblk = cap // 128

    nc = bacc.Bacc("TRN2", target_bir_lowering=False, debug=False,
                   num_devices=N_CORES, dynamic_dma_scratch_size=65536)

    ZT = nc.dram_tensor("zt", [D, N_DRUGS], bf16, kind="ExternalInput")
    ZB = nc.dram_tensor("zb", [D, BLK], bf16, kind="ExternalInput")
    WT = nc.dram_tensor("w", [D, D], bf16, kind="ExternalInput")
    DT = nc.dram_tensor("dvec", [128, KC], f32, kind="ExternalInput")
    MS = nc.dram_tensor("mask", [128, nblk, 128], bf16, kind="ExternalInput")
    IX = nc.dram_tensor("idx", [128, cap // 16], i16, kind="ExternalInput")
    OUT = nc.dram_tensor("out", [128, nblk], f32, kind="ExternalOutput")
    SD = nc.dram_tensor("s_scratch", [BLK, N_DRUGS], bf16)

    with tile.TileContext(nc) as tc:
        with (
            tc.tile_pool(name="big", bufs=1) as big,
            tc.tile_pool(name="sml", bufs=1) as sml,
            tc.tile_pool(name="stage", bufs=8) as stage,
            tc.tile_pool(name="psum", bufs=8, space="PSUM") as psum,
        ):
            d_sb = sml.tile([128, KC], f32)
            nc.sync.dma_start(d_sb[:], DT.ap())
            w_sb = sml.tile([128, KC, D], bf16)
            nc.sync.dma_start(w_sb[:], WT.ap().rearrange("(jc p) k -> p jc k", p=128))
            zb_sb = sml.tile([128, KC, BLK], bf16)
            nc.sync.dma_start(zb_sb[:], ZB.ap().rearrange("(kc p) m -> p kc m", p=128))
            zt_sb = big.tile([128, KC, N_DRUGS], bf16)
            nc.sync.dma_start(zt_sb[:], ZT.ap().rearrange("(kc p) n -> p kc n", p=128))
            # issue extraction-phase inputs now: they ride the SP HWDGE FIFO
            # ahead of the S stores and transfer during the matmul phase
            ix_sb = sml.tile([128, cap // 16], i16)
            nc.sync.dma_start(ix_sb[:], IX.ap())
            ms_sb = big.tile([128, nblk, 128], bf16)
            if ms_load:
                nc.sync.dma_start(ms_sb[:], MS.ap())
            else:
                nc.gpsimd.memset(ms_sb[:], 1.0)

            # U^T = z^T * d  (d is a per-partition scalar in each K chunk)
            for kc in range(KC):
                nc.vector.tensor_scalar_mul(zb_sb[:, kc, :], zb_sb[:, kc, :],
                                            d_sb[:, kc:kc + 1])
                nc.vector.tensor_scalar_mul(zt_sb[:, kc, :], zt_sb[:, kc, :],
                                            d_sb[:, kc:kc + 1])

            # A^T chunks for this core's block: a_sb[p, kc, m] = (U@W)[m, kc*128+p]
            a_sb = sml.tile([128, KC, BLK], bf16)
            for kc in range(KC):
                ps = psum.tile([128, BLK], f32, tag="ps")
                for jc in range(KC):
                    nc.tensor.matmul(ps[:], w_sb[:, jc, kc * 128:(kc + 1) * 128],
                                     zb_sb[:, jc, :],
                                     start=(jc == 0), stop=(jc == KC - 1))
                nc.scalar.copy(a_sb[:, kc, :], ps[:])

            # S block = A @ U^T, streamed to DRAM in [128, 512] tiles.
            # kc-outer over 8 PSUM banks: each lhsT slice streams 8 moving
            # tiles, cutting PE weight-reload overhead. Casts split ACT/DVE;
            # stores ride the ACT HWDGE ring, separate from the input loads.
            store_insts = []
            for mt in range(MT):
                pss = [psum.tile([128, 512], f32, tag="ps", name=f"ps_{mt}_{i}")
                       for i in range(NCH)]
                for kc in range(KC):
                    for nch in range(NCH):
                        nc.tensor.matmul(
                            pss[nch][:], a_sb[:, kc, mt * 128:(mt + 1) * 128],
                            zt_sb[:, kc, nch * 512:(nch + 1) * 512],
                            start=(kc == 0), stop=(kc == KC - 1))
                for nch in range(NCH):
                    s_sb = stage.tile([128, 512], bf16, tag="s_out")
                    if nch % 2 == 0:
                        nc.scalar.copy(s_sb[:], pss[nch][:])
                    else:
                        nc.vector.tensor_copy(s_sb[:], pss[nch][:])
                    st = nc.scalar.dma_start(
                        SD.ap()[mt * 128:(mt + 1) * 128, nch * 512:(nch + 1) * 512],
                        s_sb[:])
                    store_insts.append(st)

            # Per-edge extraction: gather 256B tokens (chunked so each
            # dma_gather fits the SWDGE descriptor ring), one-hot mask,
            # segmented reduce, sigmoid.
            g_sb = big.tile([128, nblk, 128], bf16)
            y_sb = sml.tile([128, nblk], f32)
            sd_view = SD.ap().rearrange("r (b c) -> (r b) c", c=128)
            p_sb = g_sb if inplace else big.tile([128, nblk, 128], bf16)
            CHUNK = 32  # blocks per dma_gather = 4096 indices
            for b0 in range(0, nblk, CHUNK):
                b1 = min(b0 + CHUNK, nblk)
                nidx = (b1 - b0) * 128
                if gather_mode == "real":
                    # single_packet=False: packed-single-packet mode faults the
                    # engine above 1024 idxs (64 descriptors/engine ceiling)
                    gi = nc.gpsimd.dma_gather(
                        g_sb[:, b0:b1, :], sd_view,
                        ix_sb[:, b0 * 8:b1 * 8],
                        num_idxs=nidx, num_idxs_reg=nidx, elem_size=128,
                        single_packet=False)
                    if dep_mode == "helper":
                        for st in store_insts:
                            add_dep_helper(gi.ins, st.ins,
                                           reason="gather reads S scratch")
                else:
                    nc.gpsimd.memset(g_sb[:, b0:b1, :], 0.5)
                if not tail:
                    continue
                nc.vector.tensor_tensor(p_sb[:, b0:b1, :], g_sb[:, b0:b1, :],
                                        ms_sb[:, b0:b1, :],
                                        op=mybir.AluOpType.mult)
                nc.vector.tensor_reduce(y_sb[:, b0:b1], p_sb[:, b0:b1, :],
                                        axis=mybir.AxisListType.X,
                                        op=mybir.AluOpType.add)
            o_sb = sml.tile([128, nblk], f32)
            if tail:
                nc.scalar.activation(o_sb[:], y_sb[:],
                                     mybir.ActivationFunctionType.Sigmoid)
            else:
                nc.vector.tensor_copy(o_sb[:], g_sb[:, :, 0])
            nc.sync.dma_start(OUT.ap(), o_sb[:])

    nc.compile()
    return nc


def _get_program(cap):
    if cap not in _cache:
        _cache[cap] = _build(cap)
    return _cache[cap]


def kernel(z_drug, global_weight, local_diag, batch_edges, edge_sub_type_idx,
           **_unused):
    from concourse.bass_utils import run_bass_kernel_spmd

    z = np.asarray(z_drug, np.float32)
    W = np.asarray(global_weight, np.float32)
    ld = np.asarray(local_diag, np.float32)
    e = np.asarray(batch_edges)
    sub = int(np.asarray(edge_sub_type_idx))
    d = ld[sub]
    assert z.shape == (N_DRUGS, D) and W.shape == (D, D)
    B = e.shape[1]
    e0 = e[0].astype(np.int64)
    e1 = e[1].astype(np.int64)

    zT = np.ascontiguousarray(z.T).astype(BF)          # [512, 4096]
    Wb = W.astype(BF)
    dT = np.ascontiguousarray(d.reshape(KC, 128).T)    # [128, 4] f32

    core = e0 // BLK
    counts = np.bincount(core, minlength=N_CORES)
    cap = max(128, int(-(-counts.max() // 128)) * 128)
    nblk = cap // 128

    in_maps = []
    positions = []
    one = BF(1.0)
    for c in range(N_CORES):
        sel = np.nonzero(core == c)[0]
        r = e0[sel] - c * BLK
        n = e1[sel]
        npad = cap - sel.size
        tok = np.zeros(cap, np.int16)
        tok[:sel.size] = (r * TPB + (n >> 7)).astype(np.int16)
        nm = np.zeros(cap, np.int64)
        nm[:sel.size] = n & 127
        # idx wrapped over 16 partitions, replicated to all 8 Q7 cores
        ixw = np.ascontiguousarray(
            np.tile(tok.reshape(cap // 16, 16).T, (8, 1)))
        mask = np.zeros((128, nblk, 128), BF)
        j = np.arange(cap)
        mask[j % 128, j // 128, nm] = one
        zB = np.ascontiguousarray(zT[:, c * BLK:(c + 1) * BLK])
        in_maps.append({"zt": zT, "zb": zB, "w": Wb, "dvec": dT,
                        "mask": mask, "idx": ixw})
        positions.append(sel)

    nc = _get_program(cap)
    res = run_bass_kernel_spmd(nc, in_maps, list(range(N_CORES)))

    out = np.empty(B, np.float32)
    for c in range(N_CORES):
        oc = np.asarray(res.results[c]["out"], np.float32)  # [128, nblk]
        flat = oc.T.reshape(-1)                             # j = b*128 + p
        out[positions[c]] = flat[:positions[c].size]
    return out


if __name__ == "__main__":
    dat = np.load("/root/problem/cached_io.npz")
    inputs = {k: dat[k] for k in ("z_drug", "global_weight", "local_diag",
                                  "batch_edges", "edge_sub_type_idx")}
    expected = dat["expected"]
    actual = kernel(**inputs)
    err = np.abs(actual - expected)
    print("max abs err:", err.max(), "mean:", err.mean())
    print("Relative error:", err.max() / np.abs(expected).max())



# revision 6
# speedup vs baseline: 2.4114x; 2.4114x over previous
"""Trainium2 Bass kernel for the Dedicom decoder problem.

Math: score_b = (z[e0]*d) @ W @ (z[e1]*d) = z[e0] @ (diag(d) W diag(d)) @ z[e1].
All-pairs scores S = (Z @ diag(d) W diag(d)) @ Z^T contain every edge score.
Each core computes 512 rows of S (its e0 block):
  A  = Z_blk @ (diag(d) W)          bf16 matmuls         [512, 512]
  S  = (A * d * 128) @ Z^T          fp8e4 DoubleRow mm   [512, 4096] (scaled 2^7)
S lands in SBUF as a bf16 table [128 part, 4 mt, 4096].  Per-edge extraction
runs on the gpsimd engine: one indirect_copy per mt row-tile gathers each
edge's f32-aligned bf16 PAIR from the f32-bitcast view of the table (per-16-
partition-group index lists, built on host).  ACT applies sigmoid(x/128) to
the bf16 view of the gathered granules; the host picks each edge's half of
its pair by pure indexing and scatters to the original edge order.
"""

import numpy as np
import ml_dtypes

BF = ml_dtypes.bfloat16
F8 = ml_dtypes.float8_e4m3

N_DRUGS = 4096
D = 512
N_CORES = 8
BLK = N_DRUGS // N_CORES   # 512 rows of S per core
MT = BLK // 128            # 4 row tiles
SSCALE = 128.0             # 2^7 fp8 dynamic-range scale on A*d

_cache = {}


def _build(I):
    """Build + compile the SPMD program for per-(mt,group) slot capacity I."""
    import concourse.bass as bass  # noqa: F401
    import concourse.bacc as bacc
    import concourse.mybir as mybir
    import concourse.tile as tile

    f32 = mybir.dt.float32
    bf16 = mybir.dt.bfloat16
    fp8 = mybir.dt.float8e4
    u16 = mybir.dt.uint16
    DR = mybir.MatmulPerfMode.DoubleRow
    SIG = mybir.ActivationFunctionType.Sigmoid
    IW = I // 16

    nc = bacc.Bacc("TRN2", target_bir_lowering=False, debug=False,
                   num_devices=N_CORES)

    ZT8 = nc.dram_tensor("zt8", [128, 2, 2, N_DRUGS], fp8, kind="ExternalInput")
    WT = nc.dram_tensor("w", [128, 4, D], bf16, kind="ExternalInput")
    ZB = nc.dram_tensor("zb", [128, 4, BLK], bf16, kind="ExternalInput")
    DV = nc.dram_tensor("dv", [128, 4], f32, kind="ExternalInput")
    DV2 = nc.dram_tensor("dv2", [128, 4], f32, kind="ExternalInput")
    IX = nc.dram_tensor("idx", [128, MT * IW], u16, kind="ExternalInput")
    OUT = nc.dram_tensor("out", [128, MT, 2 * I], bf16, kind="ExternalOutput")

    with tile.TileContext(nc) as tc:
        with (
            tc.tile_pool(name="sb", bufs=1) as sb,
            tc.tile_pool(name="psum", bufs=4, space="PSUM") as psum,
        ):
            w_sb = sb.tile([128, 4, D], bf16)
            nc.sync.dma_start(w_sb[:], WT.ap())
            zb_sb = sb.tile([128, 4, BLK], bf16)
            nc.sync.dma_start(zb_sb[:], ZB.ap())
            dv_sb = sb.tile([128, 4], f32)
            nc.sync.dma_start(dv_sb[:], DV.ap())
            dv2_sb = sb.tile([128, 4], f32)
            nc.sync.dma_start(dv2_sb[:], DV2.ap())
            ix_sb = sb.tile([128, MT * IW], u16)
            nc.sync.dma_start(ix_sb[:], IX.ap())
            zt_sb = sb.tile([128, 2, 2, N_DRUGS], fp8)
            nc.sync.dma_start(zt_sb[:], ZT8.ap())

            # W' = diag(d) @ W   (row k scaled by d[k]; k = jc*128+p)
            for jc in range(4):
                nc.vector.tensor_scalar_mul(w_sb[:, jc, :], w_sb[:, jc, :],
                                            dv_sb[:, jc:jc + 1])

            # A-phase (bf16): pa[kc][x, m] = A[m, kc*128+x],  A = Z_blk @ W'
            # a8[p, kc2, i, m] = A[m, k]*d[k]*128  (fp8e4), k = kc2*256+i*128+p
            a8 = sb.tile([128, 2, 2, BLK], fp8)
            for kch in range(2):
                pa = psum.tile([128, 1024], f32, tag="ps", name=f"pa{kch}")
                for kc in (2 * kch, 2 * kch + 1):
                    for jc in range(4):
                        nc.tensor.matmul(pa[:, (kc % 2) * 512:(kc % 2 + 1) * 512],
                                         w_sb[:, jc, kc * 128:(kc + 1) * 128],
                                         zb_sb[:, jc, :],
                                         start=(jc == 0), stop=(jc == 3))
                for kc in (2 * kch, 2 * kch + 1):
                    nc.scalar.mul(a8[:, kc // 2, kc % 2, :],
                                  pa[:, (kc % 2) * 512:(kc % 2 + 1) * 512],
                                  dv2_sb[:, kc:kc + 1])

            # S-phase (fp8 DoubleRow): s_sb[p, mt, n] = S[mt*128+p, n] bf16
            s_sb = sb.tile([128, MT, N_DRUGS], bf16)
            x_sb = sb.tile([128, MT, I], f32)
            y_sb = sb.tile([128, MT, 2 * I], bf16)
            for mt in range(4):
                for qt in range(4):
                    ps = psum.tile([128, 1024], f32, tag="ps",
                                   name=f"s_{mt}_{qt}")
                    for nch in range(2 * qt, 2 * qt + 2):
                        for kc2 in range(2):
                            nc.tensor.matmul(
                                ps[:, (nch % 2) * 512:(nch % 2 + 1) * 512],
                                a8[:, kc2, :, mt * 128:(mt + 1) * 128],
                                zt_sb[:, kc2, :, nch * 512:(nch + 1) * 512],
                                start=(kc2 == 0), stop=(kc2 == 1),
                                perf_mode=DR)
                    dst = s_sb[:, mt, qt * 1024:(qt + 1) * 1024]
                    if qt % 2 == 0:
                        nc.vector.tensor_copy(dst, ps[:])
                    else:
                        nc.scalar.copy(dst, ps[:])

                # per-edge gather: X[p, mt, i] = f32-view(S row tile)[idx]
                nc.gpsimd.indirect_copy(
                    x_sb[:, mt, :], s_sb[:, mt, :].bitcast(f32),
                    ix_sb[:, mt * IW:(mt + 1) * IW], True)
                nc.scalar.activation(y_sb[:, mt, :],
                                     x_sb[:, mt, :].bitcast(bf16),
                                     SIG, scale=1.0 / SSCALE)
                nc.scalar.dma_start(OUT.ap()[:, mt, :], y_sb[:, mt, :])

    nc.compile()
    return nc


def _get_program(I):
    if I not in _cache:
        _cache[I] = _build(I)
    return _cache[I]


def kernel(z_drug, global_weight, local_diag, batch_edges, edge_sub_type_idx,
           **_unused):
    from concourse.bass_utils import run_bass_kernel_spmd

    z = np.asarray(z_drug, np.float32)
    W = np.asarray(global_weight, np.float32)
    ld = np.asarray(local_diag, np.float32)
    e = np.asarray(batch_edges)
    sub = int(np.asarray(edge_sub_type_idx))
    d = ld[sub]
    B = e.shape[1]
    e0 = e[0].astype(np.int64)
    e1 = e[1].astype(np.int64)

    # shared (core-independent) input tensors
    zt8 = np.ascontiguousarray(
        z.T.reshape(2, 2, 128, N_DRUGS).transpose(2, 0, 1, 3)).astype(F8)
    wt = np.ascontiguousarray(
        W.reshape(4, 128, D).transpose(1, 0, 2)).astype(BF)
    dv = np.ascontiguousarray(d.reshape(4, 128).T).astype(np.float32)
    dv2 = dv * np.float32(SSCALE)

    core = e0 // BLK
    r = e0 - core * BLK
    p = r % 128
    mt = r // 128
    g = p // 16

    # slot capacity I: max edges per (core, mt, group), multiple of 16
    cell = ((core * MT + mt) * 8 + g).astype(np.int64)
    counts = np.bincount(cell, minlength=N_CORES * MT * 8)
    I = max(32, int(-(-counts.max() // 32)) * 32)
    IW = I // 16

    # slot index within each (core, mt, group) cell, in edge order
    order = np.argsort(cell, kind="stable")
    slot = np.empty(B, np.int64)
    arange = np.arange(B, dtype=np.int64)
    cs = np.concatenate([[0], np.cumsum(counts)])
    slot[order] = arange - cs[cell[order]]

    gran = (e1 >> 1).astype(np.uint16)   # f32-granule index within mt slice
    half = (e1 & 1).astype(np.int64)     # which bf16 half of the granule

    in_maps = []
    for c in range(N_CORES):
        m = core == c
        idx = np.zeros((128, MT * IW), np.uint16)
        # idx[16g+q, mt*IW + s] = granule of cell-(mt,g) slot (s*16+q)
        q = slot[m] % 16
        s = slot[m] // 16
        idx[16 * g[m] + q, mt[m] * IW + s] = gran[m]
        zb = np.ascontiguousarray(
            z[c * BLK:(c + 1) * BLK].T.reshape(4, 128, BLK)
            .transpose(1, 0, 2)).astype(BF)
        in_maps.append({"zt8": zt8, "w": wt, "zb": zb, "dv": dv, "dv2": dv2,
                        "idx": idx})

    nc = _get_program(I)
    res = run_bass_kernel_spmd(nc, in_maps, list(range(N_CORES)))

    out = np.empty(B, np.float32)
    for c in range(N_CORES):
        m = core == c
        Y = np.asarray(res.results[c]["out"])  # [128, MT, 2I] bf16
        out[m] = Y[p[m], mt[m], 2 * slot[m] + half[m]].astype(np.float32)
    return out


if __name__ == "__main__":
    dat = np.load("/root/problem/cached_io.npz")
    inputs = {k: dat[k] for k in ("z_drug", "global_weight", "local_diag",
                                  "batch_edges", "edge_sub_type_idx")}
    expected = dat["expected"]
    actual = kernel(**inputs)
    err = np.abs(actual - expected)
    print("max abs err:", err.max(), "mean:", err.mean())
    print("Relative error:", err.max() / np.abs(expected).max())


# revision 9
# speedup vs baseline: 2.9039x; 1.2042x over previous
"""Trainium2 Bass kernel for the Dedicom decoder problem.

Math: score_b = (z[e0]*d) @ W @ (z[e1]*d) = z[e0] @ (diag(d) W diag(d)) @ z[e1].
All-pairs scores S = (Z @ diag(d) W diag(d)) @ Z^T contain every edge score.
Each core computes 512 rows of S (its e0 block):
  A  = Z_blk @ (diag(d) W)          bf16 matmuls         [512, 512]
  S  = (A * d * 128) @ Z^T          fp8e4 DoubleRow mm   [512, 4096] (scaled 2^7)
S lands in SBUF as a bf16 table [128 part, 4 mt, 4096].  Per-edge extraction
runs on the gpsimd engine: one indirect_copy per mt row-tile gathers each
edge's f32-aligned bf16 PAIR from the f32-bitcast view of the table (per-16-
partition-group index lists, built on host).  ACT applies sigmoid(x/128) to
the bf16 view of the gathered granules; the host picks each edge's half of
its pair by pure indexing and scatters to the original edge order.
"""

import numpy as np
import ml_dtypes

BF = ml_dtypes.bfloat16
F8 = ml_dtypes.float8_e4m3

N_DRUGS = 4096
D = 512
N_CORES = 8
BLK = N_DRUGS // N_CORES   # 512 rows of S per core
MT = BLK // 128            # 4 row tiles
SSCALE = 128.0             # 2^7 fp8 dynamic-range scale on A*d

_cache = {}


def _build(I):
    """Build + compile the SPMD program for per-(mt,group) slot capacity I."""
    import concourse.bass as bass  # noqa: F401
    import concourse.bacc as bacc
    import concourse.mybir as mybir
    import concourse.tile as tile

    f32 = mybir.dt.float32
    bf16 = mybir.dt.bfloat16
    fp8 = mybir.dt.float8e4
    u16 = mybir.dt.uint16
    DR = mybir.MatmulPerfMode.DoubleRow
    SIG = mybir.ActivationFunctionType.Sigmoid
    IW = I // 16

    nc = bacc.Bacc("TRN2", target_bir_lowering=False, debug=False,
                   num_devices=N_CORES)

    ZT8 = nc.dram_tensor("zt8", [128, 2, 2, N_DRUGS], fp8, kind="ExternalInput")
    WT = nc.dram_tensor("w", [128, 4, D], bf16, kind="ExternalInput")
    ZB = nc.dram_tensor("zb", [128, 4, BLK], bf16, kind="ExternalInput")
    DV = nc.dram_tensor("dv", [128, 4], f32, kind="ExternalInput")
    DV2 = nc.dram_tensor("dv2", [128, 4], f32, kind="ExternalInput")
    IX = nc.dram_tensor("idx", [128, MT * IW], u16, kind="ExternalInput")
    OUT = nc.dram_tensor("out", [128, MT, 2 * I], bf16, kind="ExternalOutput")

    with tile.TileContext(nc) as tc:
        with (
            tc.tile_pool(name="sb", bufs=1) as sb,
            tc.tile_pool(name="psum", bufs=4, space="PSUM") as psum,
        ):
            # PE p-state warmup: dep-free matmuls on a memset tile keep the
            # clock at 2.4 GHz before the real work arrives.
            wu_sb = sb.tile([128, 512], bf16)
            nc.gpsimd.memset(wu_sb[:], 0.0)
            pw = psum.tile([128, 1024], f32, tag="ps", name="warm")
            for i in range(10):
                nc.tensor.matmul(pw[:, :512], wu_sb[:, :128], wu_sb[:],
                                 start=True, stop=True)

            w_sb = sb.tile([128, 4, D], bf16)
            nc.sync.dma_start(w_sb[:], WT.ap())
            dv_sb = sb.tile([128, 4], f32)
            nc.sync.dma_start(dv_sb[:], DV.ap())
            zb_sb = sb.tile([128, 4, BLK], bf16)
            nc.sync.dma_start(zb_sb[:], ZB.ap())
            dv2_sb = sb.tile([128, 4], f32)
            nc.sync.dma_start(dv2_sb[:], DV2.ap())
            ix_sb = sb.tile([128, MT * IW], u16)
            nc.sync.dma_start(ix_sb[:], IX.ap())
            zt_sb = sb.tile([128, 2, 2, N_DRUGS], fp8)
            nc.sync.dma_start(zt_sb[:], ZT8.ap())

            # W' = diag(d) @ W   (row k scaled by d[k]; k = jc*128+p)
            for jc in range(4):
                nc.vector.tensor_scalar_mul(w_sb[:, jc, :], w_sb[:, jc, :],
                                            dv_sb[:, jc:jc + 1])

            # A-phase (bf16): pa[kc][x, m] = A[m, kc*128+x],  A = Z_blk @ W'
            # a8[p, kc2, i, m] = A[m, k]*d[k]*128  (fp8e4), k = kc2*256+i*128+p
            a8 = sb.tile([128, 2, 2, BLK], fp8)
            for kch in range(2):
                pa = psum.tile([128, 1024], f32, tag="ps", name=f"pa{kch}")
                for kc in (2 * kch, 2 * kch + 1):
                    for jc in range(4):
                        nc.tensor.matmul(pa[:, (kc % 2) * 512:(kc % 2 + 1) * 512],
                                         w_sb[:, jc, kc * 128:(kc + 1) * 128],
                                         zb_sb[:, jc, :],
                                         start=(jc == 0), stop=(jc == 3))
                for kc in (2 * kch, 2 * kch + 1):
                    nc.scalar.mul(a8[:, kc // 2, kc % 2, :],
                                  pa[:, (kc % 2) * 512:(kc % 2 + 1) * 512],
                                  dv2_sb[:, kc:kc + 1])

            # S-phase (fp8 DoubleRow): s_sb[p, mt, n] = S[mt*128+p, n] bf16
            s_sb = sb.tile([128, MT, N_DRUGS], bf16)
            x_sb = sb.tile([128, MT, I], f32)
            y_sb = sb.tile([128, MT, 2 * I], bf16)
            for mt in range(4):
                for qt in range(4):
                    ps = psum.tile([128, 1024], f32, tag="ps",
                                   name=f"s_{mt}_{qt}")
                    for nch in range(2 * qt, 2 * qt + 2):
                        for kc2 in range(2):
                            nc.tensor.matmul(
                                ps[:, (nch % 2) * 512:(nch % 2 + 1) * 512],
                                a8[:, kc2, :, mt * 128:(mt + 1) * 128],
                                zt_sb[:, kc2, :, nch * 512:(nch + 1) * 512],
                                start=(kc2 == 0), stop=(kc2 == 1),
                                perf_mode=DR)
                    dst = s_sb[:, mt, qt * 1024:(qt + 1) * 1024]
                    k = mt * 4 + qt
                    if qt == 1 or k == 3:
                        nc.scalar.copy(dst, ps[:])
                    else:
                        nc.vector.tensor_copy(dst, ps[:])

                # per-edge gather: X[p, mt, i] = f32-view(S row tile)[idx]
                nc.gpsimd.indirect_copy(
                    x_sb[:, mt, :], s_sb[:, mt, :].bitcast(f32),
                    ix_sb[:, mt * IW:(mt + 1) * IW], True)
                nc.scalar.activation(y_sb[:, mt, :],
                                     x_sb[:, mt, :].bitcast(bf16),
                                     SIG, scale=1.0 / SSCALE)
                nc.scalar.dma_start(OUT.ap()[:, mt, :], y_sb[:, mt, :])

    nc.compile()
    return nc


def _get_program(I):
    if I not in _cache:
        _cache[I] = _build(I)
    return _cache[I]


def kernel(z_drug, global_weight, local_diag, batch_edges, edge_sub_type_idx,
           **_unused):
    from concourse.bass_utils import run_bass_kernel_spmd

    z = np.asarray(z_drug, np.float32)
    W = np.asarray(global_weight, np.float32)
    ld = np.asarray(local_diag, np.float32)
    e = np.asarray(batch_edges)
    sub = int(np.asarray(edge_sub_type_idx))
    d = ld[sub]
    B = e.shape[1]
    e0 = e[0].astype(np.int64)
    e1 = e[1].astype(np.int64)

    # shared (core-independent) input tensors
    zt8 = np.ascontiguousarray(
        z.T.reshape(2, 2, 128, N_DRUGS).transpose(2, 0, 1, 3)).astype(F8)
    wt = np.ascontiguousarray(
        W.reshape(4, 128, D).transpose(1, 0, 2)).astype(BF)
    dv = np.ascontiguousarray(d.reshape(4, 128).T).astype(np.float32)
    dv2 = dv * np.float32(SSCALE)

    core = e0 // BLK
    r = e0 - core * BLK
    p = r % 128
    mt = r // 128
    g = p // 16

    # slot capacity I: max edges per (core, mt, group), multiple of 16
    cell = ((core * MT + mt) * 8 + g).astype(np.int64)
    counts = np.bincount(cell, minlength=N_CORES * MT * 8)
    I = max(32, int(-(-counts.max() // 32)) * 32)
    IW = I // 16

    # slot index within each (core, mt, group) cell, in edge order
    order = np.argsort(cell, kind="stable")
    slot = np.empty(B, np.int64)
    arange = np.arange(B, dtype=np.int64)
    cs = np.concatenate([[0], np.cumsum(counts)])
    slot[order] = arange - cs[cell[order]]

    gran = (e1 >> 1).astype(np.uint16)   # f32-granule index within mt slice
    half = (e1 & 1).astype(np.int64)     # which bf16 half of the granule

    in_maps = []
    for c in range(N_CORES):
        m = core == c
        idx = np.zeros((128, MT * IW), np.uint16)
        # idx[16g+q, mt*IW + s] = granule of cell-(mt,g) slot (s*16+q)
        q = slot[m] % 16
        s = slot[m] // 16
        idx[16 * g[m] + q, mt[m] * IW + s] = gran[m]
        zb = np.ascontiguousarray(
            z[c * BLK:(c + 1) * BLK].T.reshape(4, 128, BLK)
            .transpose(1, 0, 2)).astype(BF)
        in_maps.append({"zt8": zt8, "w": wt, "zb": zb, "dv": dv, "dv2": dv2,
                        "idx": idx})

    nc = _get_program(I)
    res = run_bass_kernel_spmd(nc, in_maps, list(range(N_CORES)))

    out = np.empty(B, np.float32)
    for c in range(N_CORES):
        m = core == c
        Y = np.asarray(res.results[c]["out"])  # [128, MT, 2I] bf16
        out[m] = Y[p[m], mt[m], 2 * slot[m] + half[m]].astype(np.float32)
    return out


if __name__ == "__main__":
    dat = np.load("/root/problem/cached_io.npz")
    inputs = {k: dat[k] for k in ("z_drug", "global_weight", "local_diag",
                                  "batch_edges", "edge_sub_type_idx")}
    expected = dat["expected"]
    actual = kernel(**inputs)
    err = np.abs(actual - expected)
    print("max abs err:", err.max(), "mean:", err.mean())
    print("Relative error:", err.max() / np.abs(expected).max())


# revision 11
# speedup vs baseline: 3.0866x; 1.0629x over previous
"""Trainium2 Bass kernel for the Dedicom decoder problem.

Math: score_b = (z[e0]*d) @ W @ (z[e1]*d) = z[e0] @ (diag(d) W diag(d)) @ z[e1].
All-pairs scores S = (Z @ diag(d) W diag(d)) @ Z^T contain every edge score.
Each core computes 512 rows of S (its e0 block):
  A  = Z_blk @ (diag(d) W)          bf16 matmuls         [512, 512]
  S  = (A * d * 128) @ Z^T          fp8e4 DoubleRow mm   [512, 4096] (scaled 2^7)
S lands in SBUF as a bf16 table [128 part, 4 mt, 4096].  Per-edge extraction
runs on the gpsimd engine: one indirect_copy per mt row-tile gathers each
edge's f32-aligned bf16 PAIR from the f32-bitcast view of the table (per-16-
partition-group index lists, built on host).  ACT applies sigmoid(x/128) to
the bf16 view of the gathered granules; the host picks each edge's half of
its pair by pure indexing and scatters to the original edge order.
"""

import numpy as np
import ml_dtypes

BF = ml_dtypes.bfloat16
F8 = ml_dtypes.float8_e4m3

N_DRUGS = 4096
D = 512
N_CORES = 8
BLK = N_DRUGS // N_CORES   # 512 rows of S per core
MT = BLK // 128            # 4 row tiles
SSCALE = 128.0             # 2^7 fp8 dynamic-range scale on A*d

_cache = {}


def _build(I):
    """Build + compile the SPMD program for per-(mt,group) slot capacity I."""
    import concourse.bass as bass  # noqa: F401
    import concourse.bacc as bacc
    import concourse.mybir as mybir
    import concourse.tile as tile

    f32 = mybir.dt.float32
    bf16 = mybir.dt.bfloat16
    fp8 = mybir.dt.float8e4
    u16 = mybir.dt.uint16
    DR = mybir.MatmulPerfMode.DoubleRow
    SIG = mybir.ActivationFunctionType.Sigmoid
    IH = I // 2          # slots per (mt, column-half, group)
    IWH = IH // 16

    nc = bacc.Bacc("TRN2", target_bir_lowering=False, debug=False,
                   num_devices=N_CORES)

    ZT8 = nc.dram_tensor("zt8", [128, 2, 2, N_DRUGS], fp8, kind="ExternalInput")
    WT = nc.dram_tensor("w", [128, 4, D], bf16, kind="ExternalInput")
    ZB = nc.dram_tensor("zb", [128, 4, BLK], bf16, kind="ExternalInput")
    DV = nc.dram_tensor("dv", [128, 4], f32, kind="ExternalInput")
    DV2 = nc.dram_tensor("dv2", [128, 4], f32, kind="ExternalInput")
    IX = nc.dram_tensor("idx", [128, MT * 2 * IWH], u16, kind="ExternalInput")
    OUT = nc.dram_tensor("out", [128, MT, 2 * I], bf16, kind="ExternalOutput")

    with tile.TileContext(nc) as tc:
        with (
            tc.tile_pool(name="sb", bufs=1) as sb,
            tc.tile_pool(name="psum", bufs=4, space="PSUM") as psum,
        ):
            # PE p-state warmup: dep-free matmuls on a memset tile keep the
            # clock at 2.4 GHz before the real work arrives.
            wu_sb = sb.tile([128, 512], bf16)
            nc.gpsimd.memset(wu_sb[:], 0.0)
            pw = psum.tile([128, 1024], f32, tag="ps", name="warm")
            for i in range(10):
                nc.tensor.matmul(pw[:, :512], wu_sb[:, :128], wu_sb[:],
                                 start=True, stop=True)

            w_sb = sb.tile([128, 4, D], bf16)
            nc.sync.dma_start(w_sb[:], WT.ap())
            dv_sb = sb.tile([128, 4], f32)
            nc.sync.dma_start(dv_sb[:], DV.ap())
            zb_sb = sb.tile([128, 4, BLK], bf16)
            nc.sync.dma_start(zb_sb[:], ZB.ap())
            dv2_sb = sb.tile([128, 4], f32)
            nc.sync.dma_start(dv2_sb[:], DV2.ap())
            ix_sb = sb.tile([128, MT * 2 * IWH], u16)
            nc.sync.dma_start(ix_sb[:], IX.ap())
            zt_sb = sb.tile([128, 2, 2, N_DRUGS], fp8)
            nc.sync.dma_start(zt_sb[:, :, :, :2048], ZT8.ap()[:, :, :, :2048])
            nc.sync.dma_start(zt_sb[:, :, :, 2048:], ZT8.ap()[:, :, :, 2048:])

            # W' = diag(d) @ W   (row k scaled by d[k]; k = jc*128+p)
            for jc in range(4):
                nc.vector.tensor_scalar_mul(w_sb[:, jc, :], w_sb[:, jc, :],
                                            dv_sb[:, jc:jc + 1])

            # A-phase (bf16): pa[kc][x, m] = A[m, kc*128+x],  A = Z_blk @ W'
            # a8[p, kc2, i, m] = A[m, k]*d[k]*128  (fp8e4), k = kc2*256+i*128+p
            a8 = sb.tile([128, 2, 2, BLK], fp8)
            for kch in range(2):
                pa = psum.tile([128, 1024], f32, tag="ps", name=f"pa{kch}")
                for kc in (2 * kch, 2 * kch + 1):
                    for jc in range(4):
                        nc.tensor.matmul(pa[:, (kc % 2) * 512:(kc % 2 + 1) * 512],
                                         w_sb[:, jc, kc * 128:(kc + 1) * 128],
                                         zb_sb[:, jc, :],
                                         start=(jc == 0), stop=(jc == 3))
                for kc in (2 * kch, 2 * kch + 1):
                    nc.scalar.mul(a8[:, kc // 2, kc % 2, :],
                                  pa[:, (kc % 2) * 512:(kc % 2 + 1) * 512],
                                  dv2_sb[:, kc:kc + 1])

            # S-phase (fp8 DoubleRow): s_sb[p, mt, n] = S[mt*128+p, n] bf16
            s_sb = sb.tile([128, MT, N_DRUGS], bf16)
            x_sb = sb.tile([128, MT, I], f32)
            y_sb = sb.tile([128, MT, 2 * I], bf16)
            for mt in range(4):
                for qt in range(4):
                    ps = psum.tile([128, 1024], f32, tag="ps",
                                   name=f"s_{mt}_{qt}")
                    for nch in range(2 * qt, 2 * qt + 2):
                        for kc2 in range(2):
                            nc.tensor.matmul(
                                ps[:, (nch % 2) * 512:(nch % 2 + 1) * 512],
                                a8[:, kc2, :, mt * 128:(mt + 1) * 128],
                                zt_sb[:, kc2, :, nch * 512:(nch + 1) * 512],
                                start=(kc2 == 0), stop=(kc2 == 1),
                                perf_mode=DR)
                    dst = s_sb[:, mt, qt * 1024:(qt + 1) * 1024]
                    k = mt * 4 + qt
                    if qt == 1 or k == 3:
                        nc.scalar.copy(dst, ps[:])
                    else:
                        nc.vector.tensor_copy(dst, ps[:])

                # per-edge gather: one call per column half of the row tile
                for ch in range(2):
                    nc.gpsimd.indirect_copy(
                        x_sb[:, mt, ch * IH:(ch + 1) * IH],
                        s_sb[:, mt, ch * 2048:(ch + 1) * 2048].bitcast(f32),
                        ix_sb[:, (mt * 2 + ch) * IWH:(mt * 2 + ch + 1) * IWH],
                        True)
                nc.scalar.activation(y_sb[:, mt, :],
                                     x_sb[:, mt, :].bitcast(bf16),
                                     SIG, scale=1.0 / SSCALE)
                nc.scalar.dma_start(OUT.ap()[:, mt, :], y_sb[:, mt, :])

    nc.compile()
    return nc


def _get_program(I):
    if I not in _cache:
        _cache[I] = _build(I)
    return _cache[I]


def kernel(z_drug, global_weight, local_diag, batch_edges, edge_sub_type_idx,
           **_unused):
    from concourse.bass_utils import run_bass_kernel_spmd

    z = np.asarray(z_drug, np.float32)
    W = np.asarray(global_weight, np.float32)
    ld = np.asarray(local_diag, np.float32)
    e = np.asarray(batch_edges)
    sub = int(np.asarray(edge_sub_type_idx))
    d = ld[sub]
    B = e.shape[1]
    e0 = e[0].astype(np.int64)
    e1 = e[1].astype(np.int64)

    # shared (core-independent) input tensors
    zt8 = np.ascontiguousarray(
        z.T.reshape(2, 2, 128, N_DRUGS).transpose(2, 0, 1, 3)).astype(F8)
    wt = np.ascontiguousarray(
        W.reshape(4, 128, D).transpose(1, 0, 2)).astype(BF)
    dv = np.ascontiguousarray(d.reshape(4, 128).T).astype(np.float32)
    dv2 = dv * np.float32(SSCALE)

    core = e0 // BLK
    r = e0 - core * BLK
    p = r % 128
    mt = r // 128
    g = p // 16
    ch = (e1 >= 2048).astype(np.int64)

    # slot capacity IH: max edges per (core, mt, colhalf, group), mult of 16
    cell = (((core * MT + mt) * 2 + ch) * 8 + g).astype(np.int64)
    counts = np.bincount(cell, minlength=N_CORES * MT * 2 * 8)
    IH = max(32, int(-(-counts.max() // 32)) * 32)
    I = 2 * IH
    IWH = IH // 16

    # slot index within each (core, mt, group) cell, in edge order
    order = np.argsort(cell, kind="stable")
    slot = np.empty(B, np.int64)
    arange = np.arange(B, dtype=np.int64)
    cs = np.concatenate([[0], np.cumsum(counts)])
    slot[order] = arange - cs[cell[order]]

    gran = ((e1 % 2048) >> 1).astype(np.uint16)  # f32 idx within col half
    half = (e1 & 1).astype(np.int64)     # which bf16 half of the granule

    in_maps = []
    for c in range(N_CORES):
        m = core == c
        idx = np.zeros((128, MT * 2 * IWH), np.uint16)
        # idx[16g+q, (mt*2+ch)*IWH + s] = granule of the cell slot (s*16+q)
        q = slot[m] % 16
        s = slot[m] // 16
        idx[16 * g[m] + q, (mt[m] * 2 + ch[m]) * IWH + s] = gran[m]
        zb = np.ascontiguousarray(
            z[c * BLK:(c + 1) * BLK].T.reshape(4, 128, BLK)
            .transpose(1, 0, 2)).astype(BF)
        in_maps.append({"zt8": zt8, "w": wt, "zb": zb, "dv": dv, "dv2": dv2,
                        "idx": idx})

    nc = _get_program(I)
    res = run_bass_kernel_spmd(nc, in_maps, list(range(N_CORES)))

    out = np.empty(B, np.float32)
    for c in range(N_CORES):
        m = core == c
        Y = np.asarray(res.results[c]["out"])  # [128, MT, 2I] bf16
        col = 2 * (ch[m] * IH + slot[m]) + half[m]
        out[m] = Y[p[m], mt[m], col].astype(np.float32)
    return out


if __name__ == "__main__":
    dat = np.load("/root/problem/cached_io.npz")
    inputs = {k: dat[k] for k in ("z_drug", "global_weight", "local_diag",
                                  "batch_edges", "edge_sub_type_idx")}
    expected = dat["expected"]
    actual = kernel(**inputs)
    err = np.abs(actual - expected)
    print("max abs err:", err.max(), "mean:", err.mean())
    print("Relative error:", err.max() / np.abs(expected).max())
